# revision 19
# baseline (speedup 1.0000x reference)
"""Trainium2 Bass kernel for nn_Autoregression (16-state AR whitening log-prob).

Math: reference computes log_prob[b,k,t] = -0.5*(C*log(2pi) + logdet(Sigma_k)
+ es_k(t)^T Sigma_k^{-1} es_k(t)) with es = causal_conv(x, W, b).  Since
Sigma^{-1} = L^{-T} L^{-1} and es is affine in x, fold L^{-1} into the conv:
W2 = L^{-1} W, b2 = L^{-1} b, then mahalanobis = sum_c conv(x; W2, b2)^2.

Device layout (per core, T sharded 8 ways with an 8-sample left halo):
  conv as matmuls: out_psum[(koff,d), t] over 512-t chunks; contraction packed
  as (c_in x 2 time-shifts)=128 rows per step, 4 steps + a 65-row step for the
  j=8 tap with a ones-row carrying the bias.  DVE squares PSUM -> bf16 SBUF;
  a bf16 mask-matmul sums each state's 64 channels, accumulating all 8 state
  pairs into a [32 x 512] PSUM tile already in [k, t] layout; DVE applies
  -0.5 and the per-state constant; DMA out.
"""

import os

import numpy as np
import ml_dtypes

import concourse.bass as bass
import concourse.bacc as bacc_mod
import concourse.mybir as mybir
import concourse.tile as tile
from concourse.bass_utils import run_bass_kernel_spmd
from concourse.tile_rust import add_dep_helper

K = 16          # states
C = 64          # channels
T = 65536       # time
AR = 8          # ar order (kernel size AR+1)
NCORES = 8
TLOC = T // NCORES          # 8192 outputs per core
CHUNK = 512                 # t per matmul (one PSUM bank of fp32)
WAVE = 4                    # chunks per wave (PSUM banks used for conv)
NW = TLOC // (CHUNK * WAVE) # waves per core
KP = K // 2                 # state pairs
NSTEP = 5                   # contraction steps: 4 full + 1 (j=8 + bias row)

MM_DT = mybir.dt.float32r   # conv matmul dtype (fp32 storage, full-rate PE)
SQ_DT = mybir.dt.bfloat16   # squares / mask matmul dtype

_CACHE: dict = {}


def _build_program():
    nc = bacc_mod.Bacc()
    f32 = mybir.dt.float32

    # xin rows 0-63: x slice (with halo); rows 64-127: same shifted left by 1
    # (host-duplicated so each wave's xd tile loads with a single DMA — the
    # fp32r self-loading matmul only has room for 2 sync waits).
    xin = nc.declare_dram_parameter("xin", [128, TLOC + AR], MM_DT, isOutput=False)
    wts = nc.declare_dram_parameter("wts", [128, KP * NSTEP, 128], MM_DT, isOutput=False)
    masks = nc.declare_dram_parameter("masks", [128, KP, 32], SQ_DT, isOutput=False)
    biasc = nc.declare_dram_parameter("biasc", [K, 1], f32, isOutput=False)
    onesd = nc.declare_dram_parameter("onesd", [1, WAVE * CHUNK], MM_DT, isOutput=False)
    out = nc.declare_dram_parameter("out", [K, TLOC], f32, isOutput=True)

    WCOLS = WAVE * CHUNK          # 2048 outputs per wave
    XDW = WCOLS + AR              # xd cols per wave (halo for shifts 0..7)

    with tile.TileContext(nc) as tc:
        with (
            tc.tile_pool(name="singles", bufs=1) as singles,
            # one slot per wave: input DMAs never wait (no slot WAR/WAW — the
            # PSEUDO_DMA_DIRECT2D struct only fits one sync wait)
            tc.tile_pool(name="xpool", bufs=NW) as xpool,
            tc.tile_pool(name="sqpool", bufs=KP * WAVE + 2) as sqpool,
            tc.tile_pool(name="conv_ps", bufs=WAVE, space="PSUM") as conv_ps,
            tc.tile_pool(name="m_ps", bufs=2, space="PSUM") as m_ps,
            tc.tile_pool(name="obs_ps", bufs=1, space="PSUM") as obs_ps,
        ):
            # A self-loading fp32r matmul only has room for ONE sync wait, so
            # a real matmul must never be the first PE instruction to observe
            # more than one producer semaphore.  pe_observe() emits a tiny
            # N=1 "reader" matmul whose both operands come from a single
            # producer's tile — the normal dependency tracker then puts that
            # producer's wait on the reader, and later matmuls find the tick
            # already observed.  Ordering edges pin readers ahead of the next
            # real matmul.
            scratch = obs_ps.tile([2, 2], f32)
            pending = []

            def pe_observe(col):
                # 2x2 so the fp32r ISA restriction (even innermost n_step,
                # dst start_partition 0) holds
                i = nc.tensor.matmul(
                    scratch[0:2, 0:2], col, col, start=True, stop=True
                )
                pending.append(i)

            def pe_matmul(*args, **kw):
                i = nc.tensor.matmul(*args, **kw)
                while pending:
                    add_dep_helper(i.ins, pending.pop().ins, sync=False)
                return i

            w_sb = singles.tile([128, KP * NSTEP, 128], MM_DT)
            nc.sync.dma_start(out=w_sb, in_=wts[:, :, :])
            mask_sb = singles.tile([128, KP, 32], SQ_DT)
            nc.sync.dma_start(out=mask_sb, in_=masks[:, :, :])
            bias_sb = singles.tile([K, 1], f32)
            nc.sync.dma_start(out=bias_sb, in_=biasc[:, :])
            out_sb = singles.tile([K, TLOC], f32)
            # DVE observer: the first tensor_scalar must not be the first DVE
            # instruction to wait on the bias DMA (TS struct fits one wait)
            dve_scratch = singles.tile([K, 1], f32)
            nc.vector.tensor_copy(dve_scratch, bias_sb)

            for w in range(NW):
                base = w * WCOLS
                # xd: rows 0-63 = xin shifts (j even), rows 64-127 = xin
                # shifted one further (j odd).  xe: rows 0-63 = xin shift 8,
                # row 64 = ones (bias row).
                xd = xpool.tile([128, XDW], MM_DT, name="xd")
                nc.sync.dma_start(out=xd, in_=xin[:, base : base + XDW])
                xe = xpool.tile([C + 1, WCOLS], MM_DT, name="xe")
                nc.sync.dma_start(
                    out=xe[0:C, :], in_=xin[0:C, base + AR : base + AR + WCOLS]
                )
                nc.sync.dma_start(out=xe[C : C + 1, :], in_=onesd[:, :])

                m_bank = m_ps.tile([128, CHUNK], f32, name="m_bank")

                sqs = {}
                for p in range(KP):
                    ps = [
                        conv_ps.tile([128, CHUNK], f32, name="ps", tag="ps")
                        for _ in range(WAVE)
                    ]
                    for s in range(NSTEP):
                        if p == 0 and s == 0:
                            if w == 0:
                                pe_observe(w_sb[:, 0, 0:2])
                                pe_observe(mask_sb[:, 0, 0:2])
                            pe_observe(xd[:, 0:2])
                            pe_observe(xe[0:C, 0:2])
                            pe_observe(xe[C : C + 1, 0:2])
                        for c in range(WAVE):
                            u = c * CHUNK
                            if s < 4:
                                lhsT = w_sb[:, p * NSTEP + s, :]
                                rhs = xd[:, u + 2 * s : u + 2 * s + CHUNK]
                            else:
                                lhsT = w_sb[0 : C + 1, p * NSTEP + s, :]
                                rhs = xe[:, u : u + CHUNK]
                            pe_matmul(
                                ps[c], lhsT, rhs, start=(s == 0), stop=(s == 4)
                            )
                    for c in range(WAVE):
                        sq = sqpool.tile([128, CHUNK], SQ_DT, name="sq", tag="sq")
                        nc.scalar.activation(
                            sq, ps[c], mybir.ActivationFunctionType.Square
                        )
                        sqs[p, c] = sq
                # each chunk's 8 state-pair partial sums form one closed
                # accumulation group per 32-partition slice of m_bank
                pe_observe(sqs[0, 0][:, 0:2])
                for c in range(WAVE):
                    for p in range(KP):
                        pe_matmul(
                            m_bank[32 * c : 32 * c + 32, :],
                            mask_sb[:, p, :],
                            sqs[p, c],
                            start=(p == 0),
                            stop=(p == KP - 1),
                            tile_position=(0, 32 * c),
                        )
                for c in range(WAVE):
                    u = base + c * CHUNK
                    nc.vector.tensor_scalar(
                        out=out_sb[:, u : u + CHUNK],
                        in0=m_bank[32 * c : 32 * c + K, :],
                        scalar1=-0.5,
                        scalar2=bias_sb,
                        op0=mybir.AluOpType.mult,
                        op1=mybir.AluOpType.add,
                    )
                nc.sync.dma_start(
                    out=out[:, base : base + WCOLS],
                    in_=out_sb[:, base : base + WCOLS],
                )
    nc.compile()
    return nc


def _prep_host(W, b, Sigma):
    """Fold L^{-1} into conv weights; pack PE lhsT tiles, masks, constants."""
    W64 = W.astype(np.float64)
    b64 = b.astype(np.float64)
    S64 = Sigma.astype(np.float64)
    L = np.linalg.cholesky(S64)
    Li = np.linalg.inv(L)                       # [K, C, C] lower-triangular inv
    logdet = 2.0 * np.sum(np.log(np.diagonal(L, axis1=1, axis2=2)), axis=1)
    W2 = np.einsum("kdc,kcij->kdij", Li, W64)   # [K, C(d), C(ci), 9]
    b2 = np.einsum("kdc,kc->kd", Li, b64)       # [K, C]

    # lhsT weight tiles: w_np[r, p*5+s, m]; m = 64*koff + d
    #   s<4:  r = ci + 64*joff -> W2[2p+koff, d, ci, 2s+joff]
    #   s==4: r<64 -> W2[2p+koff, d, r, 8]; r==64 -> b2[2p+koff, d]; else 0
    w_np = np.zeros((128, KP * NSTEP, 128), np.float32)
    for p in range(KP):
        blk = W2[2 * p : 2 * p + 2]             # [2, 64(d), 64(ci), 9]
        for s in range(4):
            # [ci + 64*joff, 64*koff + d]
            sub = blk[:, :, :, 2 * s : 2 * s + 2]    # [k, d, ci, j]
            lhsT = np.transpose(sub, (3, 2, 0, 1)).reshape(128, 128)
            w_np[:, p * NSTEP + s, :] = lhsT
        top = np.transpose(blk[:, :, :, 8], (2, 0, 1)).reshape(C, 128)
        w_np[0:C, p * NSTEP + 4, :] = top
        w_np[C, p * NSTEP + 4, :] = b2[2 * p : 2 * p + 2].reshape(128)

    mask_np = np.zeros((128, KP, 32), np.float32)
    for p in range(KP):
        for koff in range(2):
            mask_np[64 * koff : 64 * koff + 64, p, 2 * p + koff] = 1.0
    mask_np = mask_np.astype(ml_dtypes.bfloat16)

    const = C * np.log(2.0 * np.pi) + logdet
    bias_np = (-0.5 * const).astype(np.float32).reshape(K, 1)
    return w_np, mask_np, bias_np


def _run(x, W, b, Sigma, trace=False):
    if "nc" not in _CACHE:
        _CACHE["nc"] = _build_program()
    nc = _CACHE["nc"]
    w_np, mask_np, bias_np = _prep_host(W, b, Sigma)

    # left causal pad (AR) plus one right pad col so the shifted copy of the
    # last core's slice stays in bounds
    xpad = np.pad(np.asarray(x, np.float32)[0], ((0, 0), (AR, 1)))  # [C, T+9]
    in_maps = []
    for i in range(NCORES):
        lo = xpad[:, TLOC * i : TLOC * i + TLOC + AR]
        hi = xpad[:, TLOC * i + 1 : TLOC * i + TLOC + AR + 1]
        in_maps.append(
            {
                "xin": np.ascontiguousarray(np.concatenate([lo, hi], axis=0)),
                "wts": w_np,
                "masks": mask_np,
                "biasc": bias_np,
                "onesd": np.ones((1, WAVE * CHUNK), np.float32),
            }
        )
    res = run_bass_kernel_spmd(
        nc, in_maps, core_ids=list(range(NCORES)), trace=trace
    )
    outs = [res.results[i]["out"] for i in range(NCORES)]
    full = np.concatenate(outs, axis=1)[None]   # [1, K, T]
    return full.astype(np.float32), res


def kernel(x, W, b, Sigma):
    out, _ = _run(x, W, b, Sigma, trace=bool(int(os.environ.get("BASS_TRACE", "0"))))
    return out
